# revision 8
# baseline (speedup 1.0000x reference)
# Trainium2 Bass kernel for nn_AttentionBasedDecoder.
#
# Sharding: data-parallel over batch B=32 -> 8 cores x 4 batches.
# On-chip layout is "feature-major": activations live as [d (partitions), r]
# with r = (b_local, f) so that the per-step matmuls (which contract over d)
# can stream straight from SBUF, and the per-(b,f) softmax normalizer is
# produced on the tensor engine via an all-ones matmul (sum over partitions,
# replicated to all partitions = free broadcast).
#
# All T=32 decode steps run in a single fully-unrolled kernel; all state
# stays in SBUF (no HBM traffic inside the loop).

import numpy as np
import ml_dtypes
from contextlib import ExitStack

B, F, D, T, V = 32, 512, 256, 32, 128
NCORES = 8
BL = B // NCORES          # 4 local batches
R = BL * F                # 2048 local rows
NKD = D // 128            # 2 partition tiles over d
BF16 = ml_dtypes.bfloat16

_CACHE = {}


def _position_encoding():
    pos = np.arange(F, dtype=np.float32)[:, None]
    _2i = np.arange(0, D, 2, dtype=np.float32)
    ang = pos / np.power(10000.0, _2i / D)
    pe = np.zeros((F, D), dtype=np.float32)
    pe[:, 0::2] = np.sin(ang)
    pe[:, 1::2] = np.cos(ang)
    return pe


def _build():
    import concourse.tile as tile
    from concourse import bacc, mybir

    f32 = mybir.dt.float32
    bf16 = mybir.dt.bfloat16
    AF = mybir.ActivationFunctionType
    OP = mybir.AluOpType

    nc = bacc.Bacc("TRN2", target_bir_lowering=False, debug=False)

    # ---- DRAM I/O (per-core) ----
    d_fT = nc.dram_tensor("fT", [NKD, 128, R], bf16, kind="ExternalInput")
    d_fqT = nc.dram_tensor("fqT", [NKD, 128, R], bf16, kind="ExternalInput")
    d_qT = nc.dram_tensor("qT", [NKD, 128, F], f32, kind="ExternalInput")
    d_decT = nc.dram_tensor("decT", [NKD, 128, T * BL], bf16, kind="ExternalInput")
    d_wsT = nc.dram_tensor("wsT", [NKD, 128, D], bf16, kind="ExternalInput")
    d_weT = nc.dram_tensor("weT", [NKD, 128, D], bf16, kind="ExternalInput")
    d_wfT = nc.dram_tensor("wfT", [NKD, 128, D], bf16, kind="ExternalInput")
    d_wqT = nc.dram_tensor("wqT", [NKD, 128, D], f32, kind="ExternalInput")
    d_whT = nc.dram_tensor("whT", [NKD, 128, D], bf16, kind="ExternalInput")
    d_W2 = nc.dram_tensor("W2", [6, 128, 4 * D], bf16, kind="ExternalInput")
    d_wgT = nc.dram_tensor("wgT", [4, 128, D], f32, kind="ExternalInput")
    d_ones = nc.dram_tensor("ones", [128, 128], bf16, kind="ExternalInput")
    d_ident = nc.dram_tensor("ident", [128, 128], bf16, kind="ExternalInput")
    d_idf = nc.dram_tensor("idf", [128, 128], f32, kind="ExternalInput")
    d_out = nc.dram_tensor("predT", [NKD, 128, T * BL], f32, kind="ExternalOutput")

    with tile.TileContext(nc) as tc, ExitStack() as ctx:
        sing = ctx.enter_context(tc.tile_pool(name="sing", bufs=1))
        sb2 = ctx.enter_context(tc.tile_pool(name="sb2", bufs=2))
        lstmp = ctx.enter_context(tc.tile_pool(name="lstmp", bufs=2))
        scrp = ctx.enter_context(tc.tile_pool(name="scrp", bufs=3))
        pu = ctx.enter_context(tc.tile_pool(name="pu", bufs=2, space="PSUM"))
        pe_ = ctx.enter_context(tc.tile_pool(name="pe", bufs=3, space="PSUM"))
        psm = ctx.enter_context(tc.tile_pool(name="psm", bufs=3, space="PSUM"))

        # ---- resident tensors ----
        fT = sing.tile([128, NKD, R], bf16)
        fqT = sing.tile([128, NKD, R], bf16)
        qT = sing.tile([128, NKD, F], f32)
        decT = sing.tile([128, NKD, T * BL], bf16)
        wsT = sing.tile([128, NKD, D], bf16)
        weT = sing.tile([128, NKD, D], bf16)
        wfT = sing.tile([128, NKD, D], bf16)
        wqT = sing.tile([128, NKD, D], f32)
        whT = sing.tile([128, NKD, D], bf16)
        W2 = sing.tile([128, 6, 4 * D], bf16)
        wgT = sing.tile([128, 4, D], f32)
        ones = sing.tile([128, 128], bf16)
        ident = sing.tile([128, 128], bf16)
        idf = sing.tile([128, 128], f32)

        for src, dst in [(d_fT, fT), (d_fqT, fqT), (d_qT, qT), (d_decT, decT), (d_wsT, wsT),
                         (d_weT, weT), (d_wfT, wfT), (d_wqT, wqT), (d_whT, whT),
                         (d_W2, W2), (d_wgT, wgT)]:
            for k in range(src.shape[0]):
                nc.sync.dma_start(out=dst[:, k, :], in_=src[k])
        nc.sync.dma_start(out=ones[:], in_=d_ones[:])
        nc.sync.dma_start(out=ident[:], in_=d_ident[:])
        nc.sync.dma_start(out=idf[:], in_=d_idf[:])

        baseT = sing.tile([128, NKD, R], bf16)
        qtermT = sing.tile([128, NKD, F], f32)
        sT = sing.tile([128, NKD, R], bf16)
        tinT = sing.tile([128, NKD, R], bf16)
        ET = sing.tile([128, NKD, R], bf16)
        alT = sing.tile([128, NKD, R], bf16)
        invZ = sing.tile([128, R], f32)
        invZb = sing.tile([128, R], bf16)
        CH = sing.tile([128, 4, T * BL], f32)   # rows of [c;h]^T, cols (t,b)
        cell = sing.tile([BL, D], f32)
        h_pad = sing.tile([128, D], f32)

        nc.vector.memset(sT[:], 0.0)
        nc.vector.memset(cell[:], 0.0)
        nc.vector.memset(h_pad[:], 0.0)

        hwb0 = sb2.tile([128, NKD, BL], f32, tag="hwb")
        hT0 = sb2.tile([128, NKD * BL], bf16, tag="hT")
        cpT0 = sb2.tile([128, NKD * BL], bf16, tag="cpT")
        nc.vector.memset(hwb0[:], 0.0)
        nc.vector.memset(hT0[:], 0.0)
        nc.vector.memset(cpT0[:], 0.0)

        # ---- preamble: qtermT = (q @ w_q.T)^T ; baseT = (f @ w_f.T)^T + rep(qtermT)
        for mt in range(NKD):
            ps = psm.tile([128, F], f32, tag="sm")
            for kt in range(NKD):
                nc.tensor.matmul(ps[:], wqT[:, kt, mt * 128:(mt + 1) * 128],
                                 qT[:, kt, :], start=(kt == 0), stop=(kt == NKD - 1))
            nc.vector.tensor_copy(out=qtermT[:, mt, :], in_=ps[:])
        for mt in range(NKD):
            for b in range(BL):
                pb = pu.tile([128, F], f32, tag="pu")
                for kt in range(NKD):
                    nc.tensor.matmul(pb[:], wfT[:, kt, mt * 128:(mt + 1) * 128],
                                     fT[:, kt, b * F:(b + 1) * F],
                                     start=(kt == 0), stop=(kt == NKD - 1))
                nc.vector.tensor_add(out=baseT[:, mt, b * F:(b + 1) * F],
                                     in0=pb[:], in1=qtermT[:, mt, :])

        hwb_prev, hT_prev, cpT_prev = hwb0, hT0, cpT0
        tail_prev = None  # deferred off-critical work of step t-1

        # ---- decode steps (software-pipelined emission) ----
        for t in range(T):
            tb = slice(t * BL, (t + 1) * BL)

            # -- LSTM gates. Split: x_t/h part first (independent of cp),
            #    then the cp part once cpT_prev is ready.
            lhs_noncp = [(decT[:, 0, tb], 0), (decT[:, 1, tb], 1),
                         (hT_prev[:, 0:BL], 4), (hT_prev[:, BL:2 * BL], 5)]
            lhs_cp = [(cpT_prev[:, 0:BL], 2), (cpT_prev[:, BL:2 * BL], 3)]
            pg = [psm.tile([BL, 512], f32, tag="sm", name=f"pg{t}_{i}")
                  for i in range(2)]
            for nchk in range(2):
                for i, (lhs, kt) in enumerate(lhs_noncp):
                    nc.tensor.matmul(pg[nchk][:], lhs,
                                     W2[:, kt, nchk * 512:(nchk + 1) * 512],
                                     start=(i == 0), stop=False)
            for nchk in range(2):
                for i, (lhs, kt) in enumerate(lhs_cp):
                    nc.tensor.matmul(pg[nchk][:], lhs,
                                     W2[:, kt, nchk * 512:(nchk + 1) * 512],
                                     start=False, stop=(i == 1))

            # -- LSTM cell (sigmoid(x) = 0.5*tanh(x/2)+0.5)
            tif = lstmp.tile([BL, 512], f32, tag="tif")
            tg = lstmp.tile([BL, D], f32, tag="tg")
            tho = lstmp.tile([BL, D], f32, tag="tho")
            thc = lstmp.tile([BL, D], f32, tag="thc")
            ig = lstmp.tile([BL, D], f32, tag="ig")
            nc.scalar.activation(out=tif[:], in_=pg[0][:], func=AF.Tanh, scale=0.5)
            nc.vector.tensor_scalar(out=tif[:], in0=tif[:], scalar1=0.5, scalar2=0.5,
                                    op0=OP.mult, op1=OP.add)
            nc.scalar.activation(out=tg[:], in_=pg[1][:, 0:D], func=AF.Tanh)
            nc.scalar.activation(out=tho[:], in_=pg[1][:, D:2 * D], func=AF.Tanh,
                                 scale=0.5)
            nc.vector.tensor_mul(out=cell[:], in0=tif[:, D:2 * D], in1=cell[:])
            nc.vector.tensor_mul(out=ig[:], in0=tif[:, 0:D], in1=tg[:])
            nc.vector.tensor_add(out=cell[:], in0=cell[:], in1=ig[:])
            nc.scalar.activation(out=thc[:], in_=cell[:], func=AF.Tanh)
            nc.vector.tensor_scalar(out=tho[:], in0=tho[:], scalar1=0.5, scalar2=0.5,
                                    op0=OP.mult, op1=OP.add)
            nc.vector.tensor_mul(out=h_pad[0:BL, :], in0=tho[:], in1=thc[:])

            # -- h^T via padded PE transpose -> CH (fp32) + bf16 copy
            hT_t = sb2.tile([128, NKD * BL], bf16, tag="hT")
            for j in range(NKD):
                pt = psm.tile([128, 128], f32, tag="sm", name=f"pt{t}_{j}")
                nc.tensor.transpose(pt[:], h_pad[:, j * 128:(j + 1) * 128], idf[:])
                nc.vector.tensor_copy(out=CH[:, NKD + j, tb], in_=pt[:, 0:BL])
                nc.vector.tensor_copy(out=hT_t[:, j * BL:(j + 1) * BL],
                                      in_=pt[:, 0:BL])

            # -- per-batch tanh bias (h @ w_h.T)^T
            hwb_t = sb2.tile([128, NKD, BL], f32, tag="hwb")
            for mt in range(NKD):
                ph = psm.tile([128, BL], f32, tag="sm", name=f"ph{t}_{mt}")
                for kt in range(NKD):
                    nc.tensor.matmul(ph[:], whT[:, kt, mt * 128:(mt + 1) * 128],
                                     hT_t[:, kt * BL:(kt + 1) * BL],
                                     start=(kt == 0), stop=(kt == NKD - 1))
                nc.vector.tensor_copy(out=hwb_t[:, mt, :], in_=ph[:])

            # -- deferred tail of step t-1: s accumulation + c path
            if tail_prev is not None:
                _emit_tail(*tail_prev)

            # -- softmax phase, b-major so per-b chains pipeline
            cpT32 = sb2.tile([128, NKD * BL], f32, tag="cpT32")
            al_t = []
            for b in range(BL):
                rs = slice(b * F, (b + 1) * F)
                pu_b = []
                for mt in range(NKD):
                    pu_t = pu.tile([128, F], f32, tag="pu", name=f"pu{t}_{mt}_{b}")
                    nc.tensor.matmul(pu_t[:], ident[:], baseT[:, mt, rs],
                                     start=True, stop=False)
                    for kt in range(NKD):
                        nc.tensor.matmul(pu_t[:], wsT[:, kt, mt * 128:(mt + 1) * 128],
                                         sT[:, kt, rs], start=False,
                                         stop=(kt == NKD - 1))
                    pu_b.append(pu_t)
                for mt in range(NKD):
                    nc.scalar.activation(out=tinT[:, mt, rs], in_=pu_b[mt][:],
                                         func=AF.Tanh, bias=hwb_t[:, mt, b:b + 1])
                for mt2 in range(NKD):
                    pe_t = pe_.tile([128, F], f32, tag="pe", name=f"pe{t}_{mt2}_{b}")
                    for kt in range(NKD):
                        nc.tensor.matmul(pe_t[:], weT[:, kt, mt2 * 128:(mt2 + 1) * 128],
                                         tinT[:, kt, rs], start=(kt == 0),
                                         stop=(kt == NKD - 1))
                    nc.scalar.activation(out=ET[:, mt2, rs], in_=pe_t[:], func=AF.Exp)
                pz_t = pe_.tile([128, F], f32, tag="pe", name=f"pz{t}_{b}")
                for kt in range(NKD):
                    nc.tensor.matmul(pz_t[:], ones[:], ET[:, kt, rs],
                                     start=(kt == 0), stop=(kt == NKD - 1))
                nc.vector.reciprocal_approx_fast(out=invZ[:, rs], in_=pz_t[:])
                nc.vector.tensor_copy(out=invZb[:, rs], in_=invZ[:, rs])
                for mt in range(NKD):
                    nc.vector.tensor_mul(out=alT[:, mt, rs], in0=ET[:, mt, rs],
                                         in1=invZb[:, rs])
                for mt in range(NKD):
                    scr2 = scrp.tile([128, F], bf16, tag="scr",
                                     name=f"scp{t}_{mt}_{b}")
                    nc.vector.scalar_tensor_tensor(
                        out=scr2[:], in0=alT[:, mt, rs], scalar=1.0,
                        in1=fqT[:, mt, rs], op0=OP.mult, op1=OP.mult,
                        accum_out=cpT32[:, mt * BL + b:mt * BL + b + 1])
            cpT_t = sb2.tile([128, NKD * BL], bf16, tag="cpT")
            nc.vector.tensor_copy(out=cpT_t[:], in_=cpT32[:])

            def _emit_tail(tt, ttb):
                for b in range(BL):
                    rs = slice(b * F, (b + 1) * F)
                    for mt in range(NKD):
                        nc.vector.tensor_add(out=sT[:, mt, rs], in0=sT[:, mt, rs],
                                             in1=alT[:, mt, rs])
                    for mt in range(NKD):
                        col = slice(tt * BL + b, tt * BL + b + 1)
                        scr = scrp.tile([128, F], bf16, tag="scr",
                                        name=f"sc{tt}_{mt}_{b}")
                        nc.vector.tensor_mul(out=scr[:], in0=alT[:, mt, rs],
                                             in1=fT[:, mt, rs])
                        nc.scalar.activation(out=scr[:], in_=scr[:], func=AF.Copy,
                                             accum_out=CH[:, mt, col])
            tail_prev = (t, tb)

            hwb_prev, hT_prev, cpT_prev = hwb_t, hT_t, cpT_t

        _emit_tail(*tail_prev)

        # ---- final: pred^T = w_g @ [c; h]^T for all (t, b) at once ----
        predsb = sing.tile([128, NKD, T * BL], f32)
        for mt in range(NKD):
            pp = pe_.tile([128, T * BL], f32, tag="pe")
            for kt in range(4):
                nc.tensor.matmul(pp[:], wgT[:, kt, mt * 128:(mt + 1) * 128],
                                 CH[:, kt, :], start=(kt == 0), stop=(kt == 3))
            nc.vector.tensor_copy(out=predsb[:, mt, :], in_=pp[:])
            nc.sync.dma_start(out=d_out[mt], in_=predsb[:, mt, :])

    nc.compile()
    return nc


def _get_nc():
    if "nc" not in _CACHE:
        _CACHE["nc"] = _build()
    return _CACHE["nc"]


def _prep_inputs(f, decoder_input_ids, emb, w_e, w_h, w_f, w_q, w_s, w_g,
                 w_ih, w_hh):
    f = np.asarray(f, dtype=np.float32)
    ids = np.asarray(decoder_input_ids).astype(np.int64)
    emb = np.asarray(emb, dtype=np.float32)
    q = _position_encoding()                     # [F, D]
    dec = emb[ids]                               # [B, T, D]

    def t2(m):  # [D, X] -> [NKD, 128, X]
        return np.ascontiguousarray(m.reshape(NKD, 128, m.shape[1]))

    wsT = t2(np.asarray(w_s, np.float32).T).astype(BF16)
    weT = t2(np.asarray(w_e, np.float32).T).astype(BF16)
    wfT = t2(np.asarray(w_f, np.float32).T).astype(BF16)
    whT = t2(np.asarray(w_h, np.float32).T).astype(BF16)
    wqT = t2(np.asarray(w_q, np.float32).T)
    qT = t2(np.ascontiguousarray(q.T))
    w_ih = np.asarray(w_ih, np.float32)
    W2 = np.concatenate([w_ih[:, :D].T, w_ih[:, D:2 * D].T,
                         np.asarray(w_hh, np.float32).T], axis=0)  # [768, 4D]
    W2 = np.ascontiguousarray(W2.reshape(6, 128, 4 * D)).astype(BF16)
    wgT = np.ascontiguousarray(np.asarray(w_g, np.float32).T.reshape(4, 128, D))
    ones = np.ones((128, 128), dtype=BF16)
    ident = np.eye(128, dtype=np.float32).astype(BF16)
    idf = np.eye(128, dtype=np.float32)

    shared = dict(wsT=wsT, weT=weT, wfT=wfT, wqT=wqT, whT=whT, qT=qT, W2=W2,
                  wgT=wgT, ones=ones, ident=ident, idf=idf)
    in_maps = []
    for c in range(NCORES):
        fl = f[c * BL:(c + 1) * BL]              # [4, F, D]
        fT = np.ascontiguousarray(fl.transpose(2, 0, 1).reshape(D, R))
        dl = dec[c * BL:(c + 1) * BL]            # [4, T, D]
        decT = np.ascontiguousarray(dl.transpose(2, 1, 0).reshape(D, T * BL))
        m = dict(shared)
        m["fT"] = fT.reshape(NKD, 128, R).astype(BF16)
        fqT = fT.reshape(D, BL, F) + q.T[:, None, :]
        m["fqT"] = fqT.reshape(NKD, 128, R).astype(BF16)
        m["decT"] = decT.reshape(NKD, 128, T * BL).astype(BF16)
        in_maps.append(m)
    return in_maps


def kernel(f, decoder_input_ids, emb, w_e, w_h, w_f, w_q, w_s, w_g, w_ih, w_hh):
    from concourse.bass_utils import run_bass_kernel_spmd

    nc = _get_nc()
    in_maps = _prep_inputs(f, decoder_input_ids, emb, w_e, w_h, w_f, w_q, w_s,
                           w_g, w_ih, w_hh)
    res = run_bass_kernel_spmd(nc, in_maps, core_ids=list(range(NCORES)))
    out = np.empty((B, T, D), dtype=np.float32)
    for c in range(NCORES):
        arr = np.asarray(res.results[c]["predT"], np.float32).reshape(D, T, BL)
        out[c * BL:(c + 1) * BL] = arr.transpose(2, 1, 0)
    return out


# revision 9
# speedup vs baseline: 4.0419x; 4.0419x over previous
# Trainium2 Bass kernel for nn_AttentionBasedDecoder.
#
# Sharding: data-parallel over batch B=32 -> 8 cores x 4 batches.
# On-chip layout is "feature-major": activations live as [d (partitions), r]
# with r = (b_local, f) so that the per-step matmuls (which contract over d)
# can stream straight from SBUF, and the per-(b,f) softmax normalizer is
# produced on the tensor engine via an all-ones matmul (sum over partitions,
# replicated to all partitions = free broadcast).
#
# All T=32 decode steps run in a single fully-unrolled kernel; all state
# stays in SBUF (no HBM traffic inside the loop).

import numpy as np
import ml_dtypes
from contextlib import ExitStack

B, F, D, T, V = 32, 512, 256, 32, 128
NCORES = 8
BL = B // NCORES          # 4 local batches
R = BL * F                # 2048 local rows
NKD = D // 128            # 2 partition tiles over d
BF16 = ml_dtypes.bfloat16

_CACHE = {}


def _position_encoding():
    pos = np.arange(F, dtype=np.float32)[:, None]
    _2i = np.arange(0, D, 2, dtype=np.float32)
    ang = pos / np.power(10000.0, _2i / D)
    pe = np.zeros((F, D), dtype=np.float32)
    pe[:, 0::2] = np.sin(ang)
    pe[:, 1::2] = np.cos(ang)
    return pe


def _build(reps=1):
    import concourse.tile as tile
    from concourse import bacc, mybir

    f32 = mybir.dt.float32
    bf16 = mybir.dt.bfloat16
    AF = mybir.ActivationFunctionType
    OP = mybir.AluOpType

    nc = bacc.Bacc("TRN2", target_bir_lowering=False, debug=False)

    # ---- DRAM I/O (per-core) ----
    d_fT = nc.dram_tensor("fT", [NKD, 128, R], bf16, kind="ExternalInput")
    d_fqT = nc.dram_tensor("fqT", [NKD, 128, R], bf16, kind="ExternalInput")
    d_qT = nc.dram_tensor("qT", [NKD, 128, F], f32, kind="ExternalInput")
    d_decT = nc.dram_tensor("decT", [NKD, 128, T * BL], bf16, kind="ExternalInput")
    d_wsT = nc.dram_tensor("wsT", [NKD, 128, D], bf16, kind="ExternalInput")
    d_weT = nc.dram_tensor("weT", [NKD, 128, D], bf16, kind="ExternalInput")
    d_wfT = nc.dram_tensor("wfT", [NKD, 128, D], bf16, kind="ExternalInput")
    d_wqT = nc.dram_tensor("wqT", [NKD, 128, D], f32, kind="ExternalInput")
    d_whT = nc.dram_tensor("whT", [NKD, 128, D], bf16, kind="ExternalInput")
    d_W2 = nc.dram_tensor("W2", [6, 128, 4 * D], bf16, kind="ExternalInput")
    d_wgT = nc.dram_tensor("wgT", [4, 128, D], f32, kind="ExternalInput")
    d_ones = nc.dram_tensor("ones", [128, 128], bf16, kind="ExternalInput")
    d_ident = nc.dram_tensor("ident", [128, 128], bf16, kind="ExternalInput")
    d_idf = nc.dram_tensor("idf", [128, 128], f32, kind="ExternalInput")
    d_out = nc.dram_tensor("predT", [NKD, 128, T * BL], f32, kind="ExternalOutput")

    with tile.TileContext(nc) as tc, ExitStack() as ctx:
        sing = ctx.enter_context(tc.tile_pool(name="sing", bufs=1))
        sb2 = ctx.enter_context(tc.tile_pool(name="sb2", bufs=2))
        lstmp = ctx.enter_context(tc.tile_pool(name="lstmp", bufs=2))
        scrp = ctx.enter_context(tc.tile_pool(name="scrp", bufs=3))
        pu = ctx.enter_context(tc.tile_pool(name="pu", bufs=2, space="PSUM"))
        pe_ = ctx.enter_context(tc.tile_pool(name="pe", bufs=3, space="PSUM"))
        psm = ctx.enter_context(tc.tile_pool(name="psm", bufs=3, space="PSUM"))

        # ---- resident tensors ----
        fT = sing.tile([128, NKD, R], bf16)
        fqT = sing.tile([128, NKD, R], bf16)
        qT = sing.tile([128, NKD, F], f32)
        decT = sing.tile([128, NKD, T * BL], bf16)
        wsT = sing.tile([128, NKD, D], bf16)
        weT = sing.tile([128, NKD, D], bf16)
        wfT = sing.tile([128, NKD, D], bf16)
        wqT = sing.tile([128, NKD, D], f32)
        whT = sing.tile([128, NKD, D], bf16)
        W2 = sing.tile([128, 6, 4 * D], bf16)
        wgT = sing.tile([128, 4, D], f32)
        ones = sing.tile([128, 128], bf16)
        ident = sing.tile([128, 128], bf16)
        idf = sing.tile([128, 128], f32)

        for src, dst in [(d_fT, fT), (d_fqT, fqT), (d_qT, qT), (d_decT, decT), (d_wsT, wsT),
                         (d_weT, weT), (d_wfT, wfT), (d_wqT, wqT), (d_whT, whT),
                         (d_W2, W2), (d_wgT, wgT)]:
            for k in range(src.shape[0]):
                nc.sync.dma_start(out=dst[:, k, :], in_=src[k])
        nc.sync.dma_start(out=ones[:], in_=d_ones[:])
        nc.sync.dma_start(out=ident[:], in_=d_ident[:])
        nc.sync.dma_start(out=idf[:], in_=d_idf[:])

        baseT = sing.tile([128, NKD, R], bf16)
        qtermT = sing.tile([128, NKD, F], f32)
        sT = sing.tile([128, NKD, R], bf16)
        tinT = sing.tile([128, NKD, R], bf16)
        ET = sing.tile([128, NKD, R], bf16)
        alT = sing.tile([128, NKD, R], bf16)
        invZ = sing.tile([128, R], f32)
        invZb = sing.tile([128, R], bf16)
        CH = sing.tile([128, 4, T * BL], f32)   # rows of [c;h]^T, cols (t,b)
        cell = sing.tile([BL, D], f32)
        h_pad = sing.tile([128, D], f32)

        nc.vector.memset(sT[:], 0.0)
        nc.vector.memset(cell[:], 0.0)
        nc.vector.memset(h_pad[:], 0.0)

        hwb0 = sb2.tile([128, NKD, BL], f32, tag="hwb")
        hT0 = sb2.tile([128, NKD * BL], bf16, tag="hT")
        cpT0 = sb2.tile([128, NKD * BL], bf16, tag="cpT")
        nc.vector.memset(hwb0[:], 0.0)
        nc.vector.memset(hT0[:], 0.0)
        nc.vector.memset(cpT0[:], 0.0)

        # ---- preamble: qtermT = (q @ w_q.T)^T ; baseT = (f @ w_f.T)^T + rep(qtermT)
        for mt in range(NKD):
            ps = psm.tile([128, F], f32, tag="sm")
            for kt in range(NKD):
                nc.tensor.matmul(ps[:], wqT[:, kt, mt * 128:(mt + 1) * 128],
                                 qT[:, kt, :], start=(kt == 0), stop=(kt == NKD - 1))
            nc.vector.tensor_copy(out=qtermT[:, mt, :], in_=ps[:])
        for mt in range(NKD):
            for b in range(BL):
                pb = pu.tile([128, F], f32, tag="pu")
                for kt in range(NKD):
                    nc.tensor.matmul(pb[:], wfT[:, kt, mt * 128:(mt + 1) * 128],
                                     fT[:, kt, b * F:(b + 1) * F],
                                     start=(kt == 0), stop=(kt == NKD - 1))
                nc.vector.tensor_add(out=baseT[:, mt, b * F:(b + 1) * F],
                                     in0=pb[:], in1=qtermT[:, mt, :])

        hwb_prev, hT_prev, cpT_prev = hwb0, hT0, cpT0
        tail_prev = None  # deferred off-critical work of step t-1

        # ---- decode steps (software-pipelined emission) ----
        for t in range(reps * T):
            tw = t % T
            tb = slice(tw * BL, (tw + 1) * BL)

            # -- LSTM gates. Split: x_t/h part first (independent of cp),
            #    then the cp part once cpT_prev is ready.
            lhs_noncp = [(decT[:, 0, tb], 0), (decT[:, 1, tb], 1),
                         (hT_prev[:, 0:BL], 4), (hT_prev[:, BL:2 * BL], 5)]
            lhs_cp = [(cpT_prev[:, 0:BL], 2), (cpT_prev[:, BL:2 * BL], 3)]
            pg = [psm.tile([BL, 512], f32, tag="sm", name=f"pg{t}_{i}")
                  for i in range(2)]
            for nchk in range(2):
                for i, (lhs, kt) in enumerate(lhs_noncp):
                    nc.tensor.matmul(pg[nchk][:], lhs,
                                     W2[:, kt, nchk * 512:(nchk + 1) * 512],
                                     start=(i == 0), stop=False)
            for nchk in range(2):
                for i, (lhs, kt) in enumerate(lhs_cp):
                    nc.tensor.matmul(pg[nchk][:], lhs,
                                     W2[:, kt, nchk * 512:(nchk + 1) * 512],
                                     start=False, stop=(i == 1))

            # -- LSTM cell (sigmoid(x) = 0.5*tanh(x/2)+0.5)
            tif = lstmp.tile([BL, 512], f32, tag="tif")
            tg = lstmp.tile([BL, D], f32, tag="tg")
            tho = lstmp.tile([BL, D], f32, tag="tho")
            thc = lstmp.tile([BL, D], f32, tag="thc")
            ig = lstmp.tile([BL, D], f32, tag="ig")
            nc.scalar.activation(out=tif[:], in_=pg[0][:], func=AF.Tanh, scale=0.5)
            nc.vector.tensor_scalar(out=tif[:], in0=tif[:], scalar1=0.5, scalar2=0.5,
                                    op0=OP.mult, op1=OP.add)
            nc.scalar.activation(out=tg[:], in_=pg[1][:, 0:D], func=AF.Tanh)
            nc.scalar.activation(out=tho[:], in_=pg[1][:, D:2 * D], func=AF.Tanh,
                                 scale=0.5)
            nc.vector.tensor_mul(out=cell[:], in0=tif[:, D:2 * D], in1=cell[:])
            nc.vector.tensor_mul(out=ig[:], in0=tif[:, 0:D], in1=tg[:])
            nc.vector.tensor_add(out=cell[:], in0=cell[:], in1=ig[:])
            nc.scalar.activation(out=thc[:], in_=cell[:], func=AF.Tanh)
            nc.vector.tensor_scalar(out=tho[:], in0=tho[:], scalar1=0.5, scalar2=0.5,
                                    op0=OP.mult, op1=OP.add)
            nc.vector.tensor_mul(out=h_pad[0:BL, :], in0=tho[:], in1=thc[:])

            # -- h^T via padded PE transpose -> CH (fp32) + bf16 copy
            hT_t = sb2.tile([128, NKD * BL], bf16, tag="hT")
            for j in range(NKD):
                pt = psm.tile([128, 128], f32, tag="sm", name=f"pt{t}_{j}")
                nc.tensor.transpose(pt[:], h_pad[:, j * 128:(j + 1) * 128], idf[:])
                nc.vector.tensor_copy(out=CH[:, NKD + j, tb], in_=pt[:, 0:BL])
                nc.vector.tensor_copy(out=hT_t[:, j * BL:(j + 1) * BL],
                                      in_=pt[:, 0:BL])

            # -- per-batch tanh bias (h @ w_h.T)^T
            hwb_t = sb2.tile([128, NKD, BL], f32, tag="hwb")
            for mt in range(NKD):
                ph = psm.tile([128, BL], f32, tag="sm", name=f"ph{t}_{mt}")
                for kt in range(NKD):
                    nc.tensor.matmul(ph[:], whT[:, kt, mt * 128:(mt + 1) * 128],
                                     hT_t[:, kt * BL:(kt + 1) * BL],
                                     start=(kt == 0), stop=(kt == NKD - 1))
                nc.vector.tensor_copy(out=hwb_t[:, mt, :], in_=ph[:])

            # -- deferred tail of step t-1: s accumulation + c path
            if tail_prev is not None:
                _emit_tail(*tail_prev)

            # -- softmax phase, b-major so per-b chains pipeline
            cpT32 = sb2.tile([128, NKD * BL], f32, tag="cpT32")
            al_t = []
            for b in range(BL):
                rs = slice(b * F, (b + 1) * F)
                pu_b = []
                for mt in range(NKD):
                    pu_t = pu.tile([128, F], f32, tag="pu", name=f"pu{t}_{mt}_{b}")
                    nc.tensor.matmul(pu_t[:], ident[:], baseT[:, mt, rs],
                                     start=True, stop=False)
                    for kt in range(NKD):
                        nc.tensor.matmul(pu_t[:], wsT[:, kt, mt * 128:(mt + 1) * 128],
                                         sT[:, kt, rs], start=False,
                                         stop=(kt == NKD - 1))
                    pu_b.append(pu_t)
                for mt in range(NKD):
                    nc.scalar.activation(out=tinT[:, mt, rs], in_=pu_b[mt][:],
                                         func=AF.Tanh, bias=hwb_t[:, mt, b:b + 1])
                for mt2 in range(NKD):
                    pe_t = pe_.tile([128, F], f32, tag="pe", name=f"pe{t}_{mt2}_{b}")
                    for kt in range(NKD):
                        nc.tensor.matmul(pe_t[:], weT[:, kt, mt2 * 128:(mt2 + 1) * 128],
                                         tinT[:, kt, rs], start=(kt == 0),
                                         stop=(kt == NKD - 1))
                    nc.scalar.activation(out=ET[:, mt2, rs], in_=pe_t[:], func=AF.Exp)
                pz_t = pe_.tile([128, F], f32, tag="pe", name=f"pz{t}_{b}")
                for kt in range(NKD):
                    nc.tensor.matmul(pz_t[:], ones[:], ET[:, kt, rs],
                                     start=(kt == 0), stop=(kt == NKD - 1))
                nc.vector.reciprocal_approx_fast(out=invZ[:, rs], in_=pz_t[:])
                nc.vector.tensor_copy(out=invZb[:, rs], in_=invZ[:, rs])
                for mt in range(NKD):
                    nc.vector.tensor_mul(out=alT[:, mt, rs], in0=ET[:, mt, rs],
                                         in1=invZb[:, rs])
                for mt in range(NKD):
                    scr2 = scrp.tile([128, F], bf16, tag="scr",
                                     name=f"scp{t}_{mt}_{b}")
                    nc.vector.scalar_tensor_tensor(
                        out=scr2[:], in0=alT[:, mt, rs], scalar=1.0,
                        in1=fqT[:, mt, rs], op0=OP.mult, op1=OP.mult,
                        accum_out=cpT32[:, mt * BL + b:mt * BL + b + 1])
            cpT_t = sb2.tile([128, NKD * BL], bf16, tag="cpT")
            nc.vector.tensor_copy(out=cpT_t[:], in_=cpT32[:])

            def _emit_tail(tt, ttb):
                for b in range(BL):
                    rs = slice(b * F, (b + 1) * F)
                    for mt in range(NKD):
                        nc.vector.tensor_add(out=sT[:, mt, rs], in0=sT[:, mt, rs],
                                             in1=alT[:, mt, rs])
                    for mt in range(NKD):
                        col = slice((tt % T) * BL + b, (tt % T) * BL + b + 1)
                        scr = scrp.tile([128, F], bf16, tag="scr",
                                        name=f"sc{tt}_{mt}_{b}")
                        nc.vector.tensor_mul(out=scr[:], in0=alT[:, mt, rs],
                                             in1=fT[:, mt, rs])
                        nc.scalar.activation(out=scr[:], in_=scr[:], func=AF.Copy,
                                             accum_out=CH[:, mt, col])
            tail_prev = (t, tb)

            hwb_prev, hT_prev, cpT_prev = hwb_t, hT_t, cpT_t

        _emit_tail(*tail_prev)

        # ---- final: pred^T = w_g @ [c; h]^T for all (t, b) at once ----
        predsb = sing.tile([128, NKD, T * BL], f32)
        for mt in range(NKD):
            pp = pe_.tile([128, T * BL], f32, tag="pe")
            for kt in range(4):
                nc.tensor.matmul(pp[:], wgT[:, kt, mt * 128:(mt + 1) * 128],
                                 CH[:, kt, :], start=(kt == 0), stop=(kt == 3))
            nc.vector.tensor_copy(out=predsb[:, mt, :], in_=pp[:])
            nc.sync.dma_start(out=d_out[mt], in_=predsb[:, mt, :])

    nc.compile()
    return nc


def _get_nc(reps=1):
    key = f"nc{reps}"
    if key not in _CACHE:
        _CACHE[key] = _build(reps)
    return _CACHE[key]


def _prep_inputs(f, decoder_input_ids, emb, w_e, w_h, w_f, w_q, w_s, w_g,
                 w_ih, w_hh):
    f = np.asarray(f, dtype=np.float32)
    ids = np.asarray(decoder_input_ids).astype(np.int64)
    emb = np.asarray(emb, dtype=np.float32)
    q = _position_encoding()                     # [F, D]
    dec = emb[ids]                               # [B, T, D]

    def t2(m):  # [D, X] -> [NKD, 128, X]
        return np.ascontiguousarray(m.reshape(NKD, 128, m.shape[1]))

    wsT = t2(np.asarray(w_s, np.float32).T).astype(BF16)
    weT = t2(np.asarray(w_e, np.float32).T).astype(BF16)
    wfT = t2(np.asarray(w_f, np.float32).T).astype(BF16)
    whT = t2(np.asarray(w_h, np.float32).T).astype(BF16)
    wqT = t2(np.asarray(w_q, np.float32).T)
    qT = t2(np.ascontiguousarray(q.T))
    w_ih = np.asarray(w_ih, np.float32)
    W2 = np.concatenate([w_ih[:, :D].T, w_ih[:, D:2 * D].T,
                         np.asarray(w_hh, np.float32).T], axis=0)  # [768, 4D]
    W2 = np.ascontiguousarray(W2.reshape(6, 128, 4 * D)).astype(BF16)
    wgT = np.ascontiguousarray(np.asarray(w_g, np.float32).T.reshape(4, 128, D))
    ones = np.ones((128, 128), dtype=BF16)
    ident = np.eye(128, dtype=np.float32).astype(BF16)
    idf = np.eye(128, dtype=np.float32)

    shared = dict(wsT=wsT, weT=weT, wfT=wfT, wqT=wqT, whT=whT, qT=qT, W2=W2,
                  wgT=wgT, ones=ones, ident=ident, idf=idf)
    in_maps = []
    for c in range(NCORES):
        fl = f[c * BL:(c + 1) * BL]              # [4, F, D]
        fT = np.ascontiguousarray(fl.transpose(2, 0, 1).reshape(D, R))
        dl = dec[c * BL:(c + 1) * BL]            # [4, T, D]
        decT = np.ascontiguousarray(dl.transpose(2, 1, 0).reshape(D, T * BL))
        m = dict(shared)
        m["fT"] = fT.reshape(NKD, 128, R).astype(BF16)
        fqT = fT.reshape(D, BL, F) + q.T[:, None, :]
        m["fqT"] = fqT.reshape(NKD, 128, R).astype(BF16)
        m["decT"] = decT.reshape(NKD, 128, T * BL).astype(BF16)
        in_maps.append(m)
    return in_maps


def kernel(f, decoder_input_ids, emb, w_e, w_h, w_f, w_q, w_s, w_g, w_ih, w_hh):
    from concourse.bass_utils import run_bass_kernel_spmd

    nc = _get_nc()
    in_maps = _prep_inputs(f, decoder_input_ids, emb, w_e, w_h, w_f, w_q, w_s,
                           w_g, w_ih, w_hh)
    res = run_bass_kernel_spmd(nc, in_maps, core_ids=list(range(NCORES)))
    out = np.empty((B, T, D), dtype=np.float32)
    for c in range(NCORES):
        arr = np.asarray(res.results[c]["predT"], np.float32).reshape(D, T, BL)
        out[c * BL:(c + 1) * BL] = arr.transpose(2, 1, 0)
    return out


# revision 10
# speedup vs baseline: 15.9386x; 3.9433x over previous
# Trainium2 Bass kernel for nn_AttentionBasedDecoder.
#
# Sharding: data-parallel over batch B=32 -> 8 cores x 4 batches.
# On-chip layout is "feature-major": activations live as [d (partitions), r]
# with r = (b_local, f) so that the per-step matmuls (which contract over d)
# can stream straight from SBUF, and the per-(b,f) softmax normalizer is
# produced on the tensor engine via an all-ones matmul (sum over partitions,
# replicated to all partitions = free broadcast).
#
# All T=32 decode steps run in a single fully-unrolled kernel; all state
# stays in SBUF (no HBM traffic inside the loop).

import numpy as np
import ml_dtypes
from contextlib import ExitStack

B, F, D, T, V = 32, 512, 256, 32, 128
NCORES = 8
BL = B // NCORES          # 4 local batches
R = BL * F                # 2048 local rows
NKD = D // 128            # 2 partition tiles over d
BF16 = ml_dtypes.bfloat16

_CACHE = {}


def _position_encoding():
    pos = np.arange(F, dtype=np.float32)[:, None]
    _2i = np.arange(0, D, 2, dtype=np.float32)
    ang = pos / np.power(10000.0, _2i / D)
    pe = np.zeros((F, D), dtype=np.float32)
    pe[:, 0::2] = np.sin(ang)
    pe[:, 1::2] = np.cos(ang)
    return pe


def _build(reps=1):
    import concourse.tile as tile
    from concourse import bacc, mybir

    f32 = mybir.dt.float32
    bf16 = mybir.dt.bfloat16
    AF = mybir.ActivationFunctionType
    OP = mybir.AluOpType

    nc = bacc.Bacc("TRN2", target_bir_lowering=False, debug=False)

    # ---- DRAM I/O (per-core) ----
    d_fT = nc.dram_tensor("fT", [NKD, 128, R], bf16, kind="ExternalInput")
    d_fqT = nc.dram_tensor("fqT", [NKD, 128, R], bf16, kind="ExternalInput")
    d_qT = nc.dram_tensor("qT", [NKD, 128, F], f32, kind="ExternalInput")
    d_decT = nc.dram_tensor("decT", [NKD, 128, T * BL], bf16, kind="ExternalInput")
    d_wsT = nc.dram_tensor("wsT", [NKD, 128, D], bf16, kind="ExternalInput")
    d_weT = nc.dram_tensor("weT", [NKD, 128, D], bf16, kind="ExternalInput")
    d_wfT = nc.dram_tensor("wfT", [NKD, 128, D], bf16, kind="ExternalInput")
    d_wqT = nc.dram_tensor("wqT", [NKD, 128, D], f32, kind="ExternalInput")
    d_whT = nc.dram_tensor("whT", [NKD, 128, D], bf16, kind="ExternalInput")
    d_W2 = nc.dram_tensor("W2", [6, 128, 4 * D], bf16, kind="ExternalInput")
    d_wgT = nc.dram_tensor("wgT", [4, 128, D], f32, kind="ExternalInput")
    d_ones = nc.dram_tensor("ones", [128, 128], bf16, kind="ExternalInput")
    d_ident = nc.dram_tensor("ident", [128, 128], bf16, kind="ExternalInput")
    d_idf = nc.dram_tensor("idf", [128, 128], f32, kind="ExternalInput")
    d_out = nc.dram_tensor("predT", [NKD, 128, T * BL], f32, kind="ExternalOutput")

    with tile.TileContext(nc) as tc, ExitStack() as ctx:
        sing = ctx.enter_context(tc.tile_pool(name="sing", bufs=1))
        sb2 = ctx.enter_context(tc.tile_pool(name="sb2", bufs=2))
        lstmp = ctx.enter_context(tc.tile_pool(name="lstmp", bufs=2))
        scrp = ctx.enter_context(tc.tile_pool(name="scrp", bufs=3))
        pu = ctx.enter_context(tc.tile_pool(name="pu", bufs=4, space="PSUM"))
        pe_ = ctx.enter_context(tc.tile_pool(name="pe", bufs=2, space="PSUM"))
        psm = ctx.enter_context(tc.tile_pool(name="psm", bufs=2, space="PSUM"))

        # ---- resident tensors ----
        fT = sing.tile([128, NKD, R], bf16)
        fqT = sing.tile([128, NKD, R], bf16)
        qT = sing.tile([128, NKD, F], f32)
        decT = sing.tile([128, NKD, T * BL], bf16)
        wsT = sing.tile([128, NKD, D], bf16)
        weT = sing.tile([128, NKD, D], bf16)
        wfT = sing.tile([128, NKD, D], bf16)
        wqT = sing.tile([128, NKD, D], f32)
        whT = sing.tile([128, NKD, D], bf16)
        W2 = sing.tile([128, 6, 4 * D], bf16)
        wgT = sing.tile([128, 4, D], f32)
        ones = sing.tile([128, 128], bf16)
        ident = sing.tile([128, 128], bf16)
        idf = sing.tile([128, 128], f32)

        for src, dst in [(d_fT, fT), (d_fqT, fqT), (d_qT, qT), (d_decT, decT), (d_wsT, wsT),
                         (d_weT, weT), (d_wfT, wfT), (d_wqT, wqT), (d_whT, whT),
                         (d_W2, W2), (d_wgT, wgT)]:
            for k in range(src.shape[0]):
                nc.sync.dma_start(out=dst[:, k, :], in_=src[k])
        nc.sync.dma_start(out=ones[:], in_=d_ones[:])
        nc.sync.dma_start(out=ident[:], in_=d_ident[:])
        nc.sync.dma_start(out=idf[:], in_=d_idf[:])

        baseT = sing.tile([128, NKD, R], bf16)
        qtermT = sing.tile([128, NKD, F], f32)
        sT = sing.tile([128, NKD, R], bf16)
        tinT = sing.tile([128, NKD, R], bf16)
        ET = sing.tile([128, NKD, R], bf16)
        alT = sing.tile([128, NKD, R], bf16)
        invZ = sing.tile([128, R], f32)
        invZb = sing.tile([128, R], bf16)
        CH = sing.tile([128, 4, T * BL], f32)   # rows of [c;h]^T, cols (t,b)
        cell = sing.tile([BL, D], f32)
        h_pad = sing.tile([128, D], f32)

        nc.vector.memset(sT[:], 0.0)
        nc.vector.memset(cell[:], 0.0)
        nc.vector.memset(h_pad[:], 0.0)

        hwb0 = sb2.tile([128, NKD, BL], f32, tag="hwb")
        hT0 = sb2.tile([128, NKD * BL], bf16, tag="hT")
        cpT0 = sb2.tile([128, NKD * BL], bf16, tag="cpT")
        nc.vector.memset(hwb0[:], 0.0)
        nc.vector.memset(hT0[:], 0.0)
        nc.vector.memset(cpT0[:], 0.0)

        # ---- preamble: qtermT = (q @ w_q.T)^T ; baseT = (f @ w_f.T)^T + rep(qtermT)
        for mt in range(NKD):
            ps = psm.tile([128, F], f32, tag="sm")
            for kt in range(NKD):
                nc.tensor.matmul(ps[:], wqT[:, kt, mt * 128:(mt + 1) * 128],
                                 qT[:, kt, :], start=(kt == 0), stop=(kt == NKD - 1))
            nc.vector.tensor_copy(out=qtermT[:, mt, :], in_=ps[:])
        for mt in range(NKD):
            for b in range(BL):
                pb = pu.tile([128, F], f32, tag="pu")
                for kt in range(NKD):
                    nc.tensor.matmul(pb[:], wfT[:, kt, mt * 128:(mt + 1) * 128],
                                     fT[:, kt, b * F:(b + 1) * F],
                                     start=(kt == 0), stop=(kt == NKD - 1))
                nc.vector.tensor_add(out=baseT[:, mt, b * F:(b + 1) * F],
                                     in0=pb[:], in1=qtermT[:, mt, :])

        hwb_prev, hT_prev, cpT_prev = hwb0, hT0, cpT0
        tail_prev = None   # (t, alT-ready) deferred c-path work
        pu_pre = {}        # pre-emitted u-psum tiles for the next step

        def _emit_u(tt, mt, b):
            rs = slice(b * F, (b + 1) * F)
            pu_t = pu.tile([128, F], f32, tag="pu", name=f"pu{tt}_{mt}_{b}")
            nc.tensor.matmul(pu_t[:], ident[:], baseT[:, mt, rs],
                             start=True, stop=False)
            for kt in range(NKD):
                nc.tensor.matmul(pu_t[:], wsT[:, kt, mt * 128:(mt + 1) * 128],
                                 sT[:, kt, rs], start=False, stop=(kt == NKD - 1))
            return pu_t

        def _emit_ctail(tt):
            for b in range(BL):
                rs = slice(b * F, (b + 1) * F)
                for mt in range(NKD):
                    col = slice((tt % T) * BL + b, (tt % T) * BL + b + 1)
                    scr = scrp.tile([128, F], bf16, tag="scr",
                                    name=f"sc{tt}_{mt}_{b}")
                    nc.vector.tensor_mul(out=scr[:], in0=alT[:, mt, rs],
                                         in1=fT[:, mt, rs])
                    nc.scalar.activation(out=scr[:], in_=scr[:], func=AF.Copy,
                                         accum_out=CH[:, mt, col])

        # ---- decode steps (software-pipelined emission) ----
        for t in range(reps * T):
            tw = t % T
            tb = slice(tw * BL, (tw + 1) * BL)

            # -- LSTM gates, x_t/h part first (independent of cp), cp part after
            lhs_noncp = [(decT[:, 0, tb], 0), (decT[:, 1, tb], 1),
                         (hT_prev[:, 0:BL], 4), (hT_prev[:, BL:2 * BL], 5)]
            lhs_cp = [(cpT_prev[:, 0:BL], 2), (cpT_prev[:, BL:2 * BL], 3)]
            pg = [psm.tile([BL, 512], f32, tag="sm", name=f"pg{t}_{i}")
                  for i in range(2)]
            for nchk in range(2):
                for i, (lhs, kt) in enumerate(lhs_noncp):
                    nc.tensor.matmul(pg[nchk][:], lhs,
                                     W2[:, kt, nchk * 512:(nchk + 1) * 512],
                                     start=(i == 0), stop=False)
            for nchk in range(2):
                for i, (lhs, kt) in enumerate(lhs_cp):
                    nc.tensor.matmul(pg[nchk][:], lhs,
                                     W2[:, kt, nchk * 512:(nchk + 1) * 512],
                                     start=False, stop=(i == 1))

            # -- deferred c-path of step t-1 (fills ACT/DVE while gates run)
            if tail_prev is not None:
                _emit_ctail(tail_prev)

            # -- LSTM cell (sigmoid(x) = 0.5*tanh(x/2)+0.5; W2 g-block
            #    pre-scaled by 2 on host so one Tanh covers g and o gates)
            tif = lstmp.tile([BL, 512], f32, tag="tif")
            tgo = lstmp.tile([BL, 512], f32, tag="tgo")
            thc = lstmp.tile([BL, D], f32, tag="thc")
            ig = lstmp.tile([BL, D], f32, tag="ig")
            nc.scalar.activation(out=tif[:], in_=pg[0][:], func=AF.Tanh, scale=0.5)
            nc.vector.tensor_scalar(out=tif[:], in0=tif[:], scalar1=0.5, scalar2=0.5,
                                    op0=OP.mult, op1=OP.add)
            nc.scalar.activation(out=tgo[:], in_=pg[1][:], func=AF.Tanh, scale=0.5)
            nc.vector.tensor_mul(out=cell[:], in0=tif[:, D:2 * D], in1=cell[:])
            nc.vector.tensor_mul(out=ig[:], in0=tif[:, 0:D], in1=tgo[:, 0:D])
            nc.vector.tensor_add(out=cell[:], in0=cell[:], in1=ig[:])
            nc.scalar.activation(out=thc[:], in_=cell[:], func=AF.Tanh)
            nc.vector.tensor_scalar(out=tgo[:, D:2 * D], in0=tgo[:, D:2 * D],
                                    scalar1=0.5, scalar2=0.5,
                                    op0=OP.mult, op1=OP.add)
            nc.vector.tensor_mul(out=h_pad[0:BL, :], in0=tgo[:, D:2 * D], in1=thc[:])

            # -- h^T via padded PE transpose; hT copy first (hw-mm needs it)
            hT_t = sb2.tile([128, NKD * BL], bf16, tag="hT")
            pts = []
            for j in range(NKD):
                pt = psm.tile([128, 128], f32, tag="sm", name=f"pt{t}_{j}")
                nc.tensor.transpose(pt[:], h_pad[:, j * 128:(j + 1) * 128], idf[:])
                nc.vector.tensor_copy(out=hT_t[:, j * BL:(j + 1) * BL],
                                      in_=pt[:, 0:BL])
                pts.append(pt)

            # -- per-batch tanh bias (h @ w_h.T)^T
            hwb_t = sb2.tile([128, NKD, BL], f32, tag="hwb")
            for mt in range(NKD):
                ph = psm.tile([128, BL], f32, tag="sm", name=f"ph{t}_{mt}")
                for kt in range(NKD):
                    nc.tensor.matmul(ph[:], whT[:, kt, mt * 128:(mt + 1) * 128],
                                     hT_t[:, kt * BL:(kt + 1) * BL],
                                     start=(kt == 0), stop=(kt == NKD - 1))
                nc.vector.tensor_copy(out=hwb_t[:, mt, :], in_=ph[:])
            for j in range(NKD):
                nc.vector.tensor_copy(out=CH[:, NKD + j, tb], in_=pts[j][:, 0:BL])

            # -- softmax phase, b-major so per-b chains pipeline
            cpT32 = sb2.tile([128, NKD * BL], f32, tag="cpT32")
            for b in range(BL):
                rs = slice(b * F, (b + 1) * F)
                for mt in range(NKD):
                    if (t, mt, b) not in pu_pre:
                        pu_pre[(t, mt, b)] = _emit_u(t, mt, b)
                for mt in range(NKD):
                    nc.scalar.activation(out=tinT[:, mt, rs],
                                         in_=pu_pre.pop((t, mt, b))[:],
                                         func=AF.Tanh, bias=hwb_t[:, mt, b:b + 1])
                for mt2 in range(NKD):
                    pe_t = pe_.tile([128, F], f32, tag="pe", name=f"pe{t}_{mt2}_{b}")
                    for kt in range(NKD):
                        nc.tensor.matmul(pe_t[:], weT[:, kt, mt2 * 128:(mt2 + 1) * 128],
                                         tinT[:, kt, rs], start=(kt == 0),
                                         stop=(kt == NKD - 1))
                    nc.scalar.activation(out=ET[:, mt2, rs], in_=pe_t[:], func=AF.Exp)
                pz_t = pe_.tile([128, F], f32, tag="pe", name=f"pz{t}_{b}")
                for kt in range(NKD):
                    nc.tensor.matmul(pz_t[:], ones[:], ET[:, kt, rs],
                                     start=(kt == 0), stop=(kt == NKD - 1))
                nc.vector.reciprocal_approx_fast(out=invZ[:, rs], in_=pz_t[:])
                nc.vector.tensor_copy(out=invZb[:, rs], in_=invZ[:, rs])
                for mt in range(NKD):
                    nc.vector.tensor_mul(out=alT[:, mt, rs], in0=ET[:, mt, rs],
                                         in1=invZb[:, rs])
                for mt in range(NKD):
                    scr2 = scrp.tile([128, F], bf16, tag="scr",
                                     name=f"scp{t}_{mt}_{b}")
                    nc.vector.scalar_tensor_tensor(
                        out=scr2[:], in0=alT[:, mt, rs], scalar=1.0,
                        in1=fqT[:, mt, rs], op0=OP.mult, op1=OP.mult,
                        accum_out=cpT32[:, mt * BL + b:mt * BL + b + 1])
            cpT_t = sb2.tile([128, NKD * BL], bf16, tag="cpT")
            nc.vector.tensor_copy(out=cpT_t[:], in_=cpT32[:])

            # -- s += alpha (right away: unblocks next step's u-matmuls)
            for b in range(BL):
                rs = slice(b * F, (b + 1) * F)
                for mt in range(NKD):
                    nc.vector.tensor_add(out=sT[:, mt, rs], in0=sT[:, mt, rs],
                                         in1=alT[:, mt, rs])
            # -- pre-emit next step's first u-matmul groups (fills PE gap)
            if t + 1 < reps * T:
                for b in range(2):
                    for mt in range(NKD):
                        pu_pre[(t + 1, mt, b)] = _emit_u(t + 1, mt, b)

            tail_prev = t
            hwb_prev, hT_prev, cpT_prev = hwb_t, hT_t, cpT_t

        _emit_ctail(tail_prev)

        # ---- final: pred^T = w_g @ [c; h]^T for all (t, b) at once ----
        predsb = sing.tile([128, NKD, T * BL], f32)
        for mt in range(NKD):
            pp = pe_.tile([128, T * BL], f32, tag="pe")
            for kt in range(4):
                nc.tensor.matmul(pp[:], wgT[:, kt, mt * 128:(mt + 1) * 128],
                                 CH[:, kt, :], start=(kt == 0), stop=(kt == 3))
            nc.vector.tensor_copy(out=predsb[:, mt, :], in_=pp[:])
            nc.sync.dma_start(out=d_out[mt], in_=predsb[:, mt, :])

    nc.compile()
    return nc


def _get_nc(reps=1):
    key = f"nc{reps}"
    if key not in _CACHE:
        _CACHE[key] = _build(reps)
    return _CACHE[key]


def _prep_inputs(f, decoder_input_ids, emb, w_e, w_h, w_f, w_q, w_s, w_g,
                 w_ih, w_hh):
    f = np.asarray(f, dtype=np.float32)
    ids = np.asarray(decoder_input_ids).astype(np.int64)
    emb = np.asarray(emb, dtype=np.float32)
    q = _position_encoding()                     # [F, D]
    dec = emb[ids]                               # [B, T, D]

    def t2(m):  # [D, X] -> [NKD, 128, X]
        return np.ascontiguousarray(m.reshape(NKD, 128, m.shape[1]))

    wsT = t2(np.asarray(w_s, np.float32).T).astype(BF16)
    weT = t2(np.asarray(w_e, np.float32).T).astype(BF16)
    wfT = t2(np.asarray(w_f, np.float32).T).astype(BF16)
    whT = t2(np.asarray(w_h, np.float32).T).astype(BF16)
    wqT = t2(np.asarray(w_q, np.float32).T)
    qT = t2(np.ascontiguousarray(q.T))
    w_ih = np.asarray(w_ih, np.float32)
    W2 = np.concatenate([w_ih[:, :D].T, w_ih[:, D:2 * D].T,
                         np.asarray(w_hh, np.float32).T], axis=0)  # [768, 4D]
    W2[:, 2 * D:3 * D] *= 2.0  # g-gate pre-scale: one Tanh(x/2) op covers g+o
    W2 = np.ascontiguousarray(W2.reshape(6, 128, 4 * D)).astype(BF16)
    wgT = np.ascontiguousarray(np.asarray(w_g, np.float32).T.reshape(4, 128, D))
    ones = np.ones((128, 128), dtype=BF16)
    ident = np.eye(128, dtype=np.float32).astype(BF16)
    idf = np.eye(128, dtype=np.float32)

    shared = dict(wsT=wsT, weT=weT, wfT=wfT, wqT=wqT, whT=whT, qT=qT, W2=W2,
                  wgT=wgT, ones=ones, ident=ident, idf=idf)
    in_maps = []
    for c in range(NCORES):
        fl = f[c * BL:(c + 1) * BL]              # [4, F, D]
        fT = np.ascontiguousarray(fl.transpose(2, 0, 1).reshape(D, R))
        dl = dec[c * BL:(c + 1) * BL]            # [4, T, D]
        decT = np.ascontiguousarray(dl.transpose(2, 1, 0).reshape(D, T * BL))
        m = dict(shared)
        m["fT"] = fT.reshape(NKD, 128, R).astype(BF16)
        fqT = fT.reshape(D, BL, F) + q.T[:, None, :]
        m["fqT"] = fqT.reshape(NKD, 128, R).astype(BF16)
        m["decT"] = decT.reshape(NKD, 128, T * BL).astype(BF16)
        in_maps.append(m)
    return in_maps


def kernel(f, decoder_input_ids, emb, w_e, w_h, w_f, w_q, w_s, w_g, w_ih, w_hh):
    from concourse.bass_utils import run_bass_kernel_spmd

    nc = _get_nc()
    in_maps = _prep_inputs(f, decoder_input_ids, emb, w_e, w_h, w_f, w_q, w_s,
                           w_g, w_ih, w_hh)
    res = run_bass_kernel_spmd(nc, in_maps, core_ids=list(range(NCORES)))
    out = np.empty((B, T, D), dtype=np.float32)
    for c in range(NCORES):
        arr = np.asarray(res.results[c]["predT"], np.float32).reshape(D, T, BL)
        out[c * BL:(c + 1) * BL] = arr.transpose(2, 1, 0)
    return out
